# revision 1
# baseline (speedup 1.0000x reference)
"""Trainium2 Bass kernel for nn_AttentionSubModule: batched tiny attention.

Per item (131072 total): x row of 225 = 25 tokens x 9 dims, 4 token groups
each with own 9x9 Wq/Wk/Wv + bias; scores = qk^T/3 (+mask*-1e9), softmax,
out = attn@v + residual, LayerNorm over the 9-dim axis.

Mapping: pure data parallel over 8 cores (16384 items each), 128 items per
SBUF tile (items on partitions).

 - q/k/v projections on the PE: transpose x (PE transpose), multiply by
   block-diagonal per-token weight matrices (shared stationary), add biases
   during the PSUM->SBUF evacuation on the scalar engine (per-partition bias
   in the transposed layout), then PE-transpose back to item-rows.
 - scores and attn@v: vector-engine broadcast-AP multiplies + strided
   reduces (per-item 25x25x9 contractions don't map onto the PE).
 - exp on the scalar engine; softmax division folded away via LayerNorm
   scale invariance: LN(attn@v/Z + x) == LN(attn_unnorm@v + Z*x).
"""

import numpy as np
from contextlib import ExitStack

import concourse.bass as bass
import concourse.tile as tile
from concourse import mybir
from concourse.bass_utils import run_bass_kernel_spmd

KV = 9
NQ = 25
D = NQ * KV  # 225
GROUPS = [(0, 27, 3), (27, 117, 10), (117, 207, 10), (207, 225, 2)]
N_CORES = 8
P = 128
EPS = 1e-5
F32 = mybir.dt.float32
BF16 = mybir.dt.float16
SCORES_BF16 = True
ATTNV_BF16 = True

NA = 14 * KV   # chunk A: tokens 0..13 -> 126 rows
NB = 11 * KV   # chunk B: tokens 14..24 -> 99 rows

# pmat (per-partition consts) column layout:
#   [0:128)    identity 128x128
#   [128:254)  MqA  (126x126)    [254:380) MkA   [380:506) MvA
#   [506:605)  MqB  (99x99)      [605:704) MkB   [704:803) MvB
#   803 bqA | 804 bqB | 805 bkA | 806 bkB | 807 bvA | 808 bvB
PMAT_COLS = 128 + 3 * NA + 3 * NB + 6  # 809

# cst (broadcast consts): [mask 25 | gamma 9 | beta 9]
CST_LEN = NQ + KV + KV


def _bcast_ap(handle, n_part):
    ap = handle[:]
    return bass.AP(tensor=ap.tensor, offset=ap.offset, ap=[[0, n_part]] + list(ap.ap))


def build_program(b_core, probe=3):
    assert b_core % P == 0
    ntiles = b_core // P
    nc = bass.Bass("TRN2", target_bir_lowering=False)

    x_d = nc.dram_tensor("x", [b_core, D], F32, kind="ExternalInput")
    cst_d = nc.dram_tensor("cst", [CST_LEN], F32, kind="ExternalInput")
    pmat_d = nc.dram_tensor("pmat", [P, PMAT_COLS], F32, kind="ExternalInput")
    out_d = nc.dram_tensor("out", [b_core, D], F32, kind="ExternalOutput")

    with tile.TileContext(nc) as tc, ExitStack() as ctx:
        consts = ctx.enter_context(tc.tile_pool(name="consts", bufs=1))
        xin = ctx.enter_context(tc.tile_pool(name="xin", bufs=4))
        tlay = ctx.enter_context(tc.tile_pool(name="tlay", bufs=2))
        proj = ctx.enter_context(tc.tile_pool(name="proj", bufs=3))
        big = ctx.enter_context(tc.tile_pool(name="big", bufs=3))
        sm = ctx.enter_context(tc.tile_pool(name="sm", bufs=3))
        outp = ctx.enter_context(tc.tile_pool(name="outp", bufs=3))
        psum = ctx.enter_context(tc.tile_pool(name="psum", bufs=8, space="PSUM"))

        # ---- constants ----
        cst_t = consts.tile([P, CST_LEN], F32)
        nc.gpsimd.dma_start(out=cst_t, in_=_bcast_ap(cst_d, P))
        m_t = cst_t[:, 0:NQ]
        g_t = cst_t[:, NQ : NQ + KV]
        b_t = cst_t[:, NQ + KV : NQ + 2 * KV]

        pm_t = consts.tile([P, PMAT_COLS], F32)
        nc.sync.dma_start(out=pm_t, in_=pmat_d[:, :])
        ident = pm_t[:, 0:128]
        mA = {}
        mB = {}
        o = 128
        for nm in ("q", "k", "v"):
            mA[nm] = pm_t[0:NA, o : o + NA]; o += NA
        for nm in ("q", "k", "v"):
            mB[nm] = pm_t[0:NB, o : o + NB]; o += NB
        biasA = {}
        biasB = {}
        for nm in ("q", "k", "v"):
            biasA[nm] = pm_t[0:NA, o : o + 1]; o += 1
            biasB[nm] = pm_t[0:NB, o : o + 1]; o += 1
        assert o == PMAT_COLS

        # expm[p, j] = exp(-1e9 * mask[j]); multiplied into exp(scores)
        expm_t = consts.tile([P, NQ], BF16 if ATTNV_BF16 else F32)
        nc.scalar.activation(
            expm_t[:], m_t, mybir.ActivationFunctionType.Exp, bias=0.0, scale=-1e9
        )
        eps_t = consts.tile([P, 1], F32)
        nc.vector.memset(eps_t[:], EPS)
        # constant exp-shift: exp(s - 8) keeps fp16 attention weights and
        # partial sums in range (max score ~16.4); softmax is shift-invariant
        # and the LN scale-invariance absorbs the global factor exactly.
        shift_t = consts.tile([P, 1], F32)
        nc.vector.memset(shift_t[:], -8.0)
        # wait-absorbers: sync DVE/ACT on the const DMAs via tiny copies so the
        # wide TensorTensor encodings never need more than one sync-wait.
        absorb_t = consts.tile([P, 4], F32)
        nc.vector.tensor_copy(absorb_t[:], cst_t[:, 0:4])
        absorb2_t = consts.tile([P, 4], F32)
        nc.vector.tensor_copy(absorb2_t[:], pm_t[:, 0:4])

        inv_sqrt_kv = float(1.0 / np.sqrt(KV))
        AF = mybir.ActivationFunctionType

        for t in range(ntiles):
            xt = xin.tile([P, D], F32)
            nc.sync.dma_start(out=xt, in_=x_d[t * P : (t + 1) * P, :])
            xv = xt[:].rearrange("p (i d) -> p i d", i=NQ)

            if probe == 0:
                o_t0 = outp.tile([P, NQ, KV], F32, tag="o")
                nc.vector.tensor_copy(o_t0[:].rearrange("p a b -> p (a b)"), xt[:])
                nc.sync.dma_start(
                    out=out_d[t * P : (t + 1) * P, :],
                    in_=o_t0[:].rearrange("p a b -> p (a b)"),
                )
                continue
            # ---- projections on PE (transposed layout) ----
            xT1p = psum.tile([P, P], F32, tag="ps")
            xT2p = psum.tile([P, P], F32, tag="ps")
            nc.tensor.transpose(xT1p[0:NA, :], xt[:, 0:NA], ident)
            nc.tensor.transpose(xT2p[0:NB, :], xt[:, NA:D], ident)
            xT1 = tlay.tile([P, P], F32, tag="xT1")
            xT2 = tlay.tile([P, P], F32, tag="xT2")
            nc.scalar.copy(xT1[0:NA, :], xT1p[0:NA, :])
            nc.scalar.copy(xT2[0:NB, :], xT2p[0:NB, :])

            nat = {}
            for nm in ("q", "k", "v"):
                pA = psum.tile([P, P], F32, tag="ps")
                pB = psum.tile([P, P], F32, tag="ps")
                nc.tensor.matmul(pA[0:NA, :], mA[nm], xT1[0:NA, :], start=True, stop=True)
                nc.tensor.matmul(pB[0:NB, :], mB[nm], xT2[0:NB, :], start=True, stop=True)
                # evacuate with bias add (per-partition bias in T layout)
                sT1 = tlay.tile([P, P], F32, tag=f"s{nm}1")
                sT2 = tlay.tile([P, P], F32, tag=f"s{nm}2")
                nc.scalar.activation(sT1[0:NA, :], pA[0:NA, :], AF.Identity,
                                     bias=biasA[nm], scale=1.0)
                nc.scalar.activation(sT2[0:NB, :], pB[0:NB, :], AF.Identity,
                                     bias=biasB[nm], scale=1.0)
                # transpose back to item-rows
                nA = psum.tile([P, P], F32, tag="ps")
                nB_ = psum.tile([P, P], F32, tag="ps")
                nc.tensor.transpose(nA[:, 0:NA], sT1[0:NA, :], ident[0:NA, 0:NA])
                nc.tensor.transpose(nB_[:, 0:NB], sT2[0:NB, :], ident[0:NB, 0:NB])
                want_bf = (SCORES_BF16 and nm in ("q", "k")) or (ATTNV_BF16 and nm == "v")
                dst = proj.tile([P, NQ, KV], BF16 if want_bf else F32, tag=nm)
                flat = dst[:].rearrange("p a b -> p (a b)")
                nc.scalar.copy(flat[:, 0:NA], nA[:, 0:NA])
                nc.scalar.copy(flat[:, NA:D], nB_[:, 0:NB])
                nat[nm] = dst

            q_t, k_t, v_t = nat["q"], nat["k"], nat["v"]

            if probe == 1:
                o_t1 = outp.tile([P, NQ, KV], F32, tag="o")
                nc.vector.tensor_add(o_t1[:], q_t[:], k_t[:])
                nc.sync.dma_start(
                    out=out_d[t * P : (t + 1) * P, :],
                    in_=o_t1[:].rearrange("p a b -> p (a b)"),
                )
                continue

            # ---- scores + exp (no max-subtraction: |scores| <~ 15) ----
            pr2 = big.tile([P, NQ, NQ, KV], BF16 if SCORES_BF16 else F32, tag="bigprod")
            nc.vector.tensor_mul(
                pr2[:],
                q_t[:].unsqueeze(2).broadcast_to((P, NQ, NQ, KV)),
                k_t[:].unsqueeze(1).broadcast_to((P, NQ, NQ, KV)),
            )
            sc = sm.tile([P, NQ, NQ], F32, tag="sc")
            if SCORES_BF16:
                # strided TT-tree reduce over d (faster than 1x tensor_reduce)
                PD = BF16
                t1 = sm.tile([P, NQ * NQ, 4], PD, tag="sct1")
                p4 = pr2[:].rearrange("p a b d -> p (a b) d")
                nc.vector.tensor_add(t1[:], p4[:, :, 0:4], p4[:, :, 4:8])
                t2 = sm.tile([P, NQ * NQ, 2], PD, tag="sct2")
                nc.vector.tensor_add(t2[:], t1[:, :, 0:2], t1[:, :, 2:4])
                t3 = sm.tile([P, NQ * NQ, 1], PD, tag="sct3")
                nc.vector.tensor_add(t3[:], t2[:, :, 0:1], t2[:, :, 1:2])
                nc.vector.tensor_add(
                    sc[:].rearrange("p a b -> p (a b)").unsqueeze(2),
                    t3[:], p4[:, :, 8:9],
                )
            else:
                nc.vector.tensor_reduce(
                    sc[:], pr2[:], axis=mybir.AxisListType.X, op=mybir.AluOpType.add
                )
            ex = sm.tile([P, NQ, NQ], BF16 if ATTNV_BF16 else F32, tag="ex")
            nc.scalar.activation(
                ex[:].rearrange("p a b -> p (a b)"),
                sc[:].rearrange("p a b -> p (a b)"),
                AF.Exp, bias=shift_t[:], scale=inv_sqrt_kv,
            )
            # apply mask weights: e'[p,i,j] = e[p,i,j] * expm[p,j]
            nc.vector.tensor_mul(
                ex[:], ex[:], expm_t[:].unsqueeze(1).broadcast_to((P, NQ, NQ))
            )
            # Z[p, i] = sum_j e'
            z_t = sm.tile([P, NQ], F32, tag="z")
            nc.vector.tensor_reduce(
                z_t[:], ex[:], axis=mybir.AxisListType.X, op=mybir.AluOpType.add
            )

            if probe == 2:
                o_t2 = outp.tile([P, NQ, KV], F32, tag="o")
                nc.vector.tensor_mul(
                    o_t2[:], v_t[:], z_t[:].unsqueeze(2).broadcast_to((P, NQ, KV))
                )
                nc.sync.dma_start(
                    out=out_d[t * P : (t + 1) * P, :],
                    in_=o_t2[:].rearrange("p a b -> p (a b)"),
                )
                continue
            # ---- unnormalized attn @ v: un[p,i,e] = sum_j e'[p,i,j] v[p,j,e] ----
            # vE: v reordered (e-major) so the products-TT innermost stride is 1
            vE = proj.tile([P, KV, NQ], BF16 if ATTNV_BF16 else F32, tag="ve")
            nc.scalar.copy(vE[:], v_t[:].transpose([0, 2, 1]))
            pr3 = big.tile([P, NQ, KV, NQ], BF16 if ATTNV_BF16 else F32, tag="bigprod")
            nc.vector.tensor_mul(
                pr3[:],
                ex[:].unsqueeze(2).broadcast_to((P, NQ, KV, NQ)),
                vE[:].unsqueeze(1).broadcast_to((P, NQ, KV, NQ)),
            )
            u_t = outp.tile([P, NQ, KV], F32, tag="u")
            if ATTNV_BF16:
                PD = BF16
                q3 = pr3[:].rearrange("p a b d -> p (a b) d")
                r1 = sm.tile([P, NQ * KV, 12], PD, tag="avr1")
                nc.vector.tensor_add(r1[:], q3[:, :, 0:12], q3[:, :, 12:24])
                r2 = sm.tile([P, NQ * KV, 6], PD, tag="avr2")
                nc.vector.tensor_add(r2[:], r1[:, :, 0:6], r1[:, :, 6:12])
                r3 = sm.tile([P, NQ * KV, 3], PD, tag="avr3")
                nc.vector.tensor_add(r3[:], r2[:, :, 0:3], r2[:, :, 3:6])
                r4 = sm.tile([P, NQ * KV, 1], PD, tag="avr4")
                nc.vector.tensor_add(r4[:], r3[:, :, 0:1], r3[:, :, 1:2])
                r5 = sm.tile([P, NQ * KV, 1], PD, tag="avr5")
                nc.vector.tensor_add(r5[:], r4[:], r3[:, :, 2:3])
                nc.vector.tensor_add(
                    u_t[:].rearrange("p a b -> p (a b)").unsqueeze(2),
                    r5[:], q3[:, :, 24:25],
                )
            else:
                nc.vector.tensor_reduce(
                    u_t[:], pr3[:], axis=mybir.AxisListType.X, op=mybir.AluOpType.add
                )
            # u += Z * x   (residual, scaled by Z; LN is scale-invariant)
            zx = outp.tile([P, NQ, KV], F32, tag="zx")
            nc.vector.tensor_mul(
                zx[:], xv, z_t[:].unsqueeze(2).broadcast_to((P, NQ, KV))
            )
            nc.vector.tensor_add(u_t[:], u_t[:], zx[:])

            # ---- LayerNorm over e (9) ----
            s_t = sm.tile([P, NQ], F32, tag="lnsum")
            nc.vector.tensor_reduce(
                s_t[:], u_t[:], axis=mybir.AxisListType.X, op=mybir.AluOpType.add
            )
            mu = sm.tile([P, NQ], F32, tag="mu")
            nc.scalar.mul(mu[:], s_t[:], 1.0 / KV)
            cen = outp.tile([P, NQ, KV], F32, tag="cen")
            nc.vector.tensor_sub(
                cen[:], u_t[:], mu[:].unsqueeze(2).broadcast_to((P, NQ, KV))
            )
            sq = outp.tile([P, NQ, KV], F32, tag="sq")
            nc.scalar.square(
                sq[:].rearrange("p a b -> p (a b)"), cen[:].rearrange("p a b -> p (a b)")
            )
            vs = sm.tile([P, NQ], F32, tag="vs")
            nc.vector.tensor_reduce(
                vs[:], sq[:], axis=mybir.AxisListType.X, op=mybir.AluOpType.add
            )
            # scale-correct eps: u = Z*out_ref, so var_ref = var_u / Z^2;
            # rstd_ref/Z = 1/sqrt(var_u + Z^2*eps)
            zsq = sm.tile([P, NQ], F32, tag="zsq")
            nc.scalar.square(zsq[:], z_t[:])
            vs2 = sm.tile([P, NQ], F32, tag="vs2")
            nc.vector.scalar_tensor_tensor(
                vs2[:], zsq[:], float(KV * EPS), vs[:],
                op0=mybir.AluOpType.mult, op1=mybir.AluOpType.add,
            )
            sd = sm.tile([P, NQ], F32, tag="sd")
            nc.scalar.activation(
                sd[:], vs2[:], AF.Sqrt, bias=0.0, scale=1.0 / KV
            )
            rstd = sm.tile([P, NQ], F32, tag="rstd")
            nc.vector.reciprocal(rstd[:], sd[:])

            o_t = outp.tile([P, NQ, KV], F32, tag="o")
            nc.vector.tensor_mul(
                o_t[:], cen[:], rstd[:].unsqueeze(2).broadcast_to((P, NQ, KV))
            )
            nc.vector.tensor_mul(
                o_t[:], o_t[:], g_t.unsqueeze(1).broadcast_to((P, NQ, KV))
            )
            nc.vector.tensor_add(
                o_t[:], o_t[:], b_t.unsqueeze(1).broadcast_to((P, NQ, KV))
            )
            nc.sync.dma_start(
                out=out_d[t * P : (t + 1) * P, :],
                in_=o_t[:].rearrange("p a b -> p (a b)"),
            )

    _split_multi_waits(nc)
    return nc


def _split_multi_waits(nc):
    """Walrus allows only one sync-wait slot on most instruction encodings.
    Hoist excess waits into NoOps inserted just before the offender (same
    engine, same block => same ordering semantics)."""
    for f in nc.m.functions:
        for b in f.blocks:
            i = 0
            while i < len(b.instructions):
                inst = b.instructions[i]
                si = getattr(inst, "sync_info", None)
                if si is not None and si.on_wait and len(si.on_wait) > 1:
                    extra = si.on_wait[:-1]
                    si.on_wait = si.on_wait[-1:]
                    for w in extra:
                        nop = mybir.InstNoOp(
                            name=nc.get_next_instruction_name(),
                            engine=inst.engine,
                            ins=[],
                            outs=[],
                            sync_info=mybir.SyncInfo(on_wait=[w], on_update=[]),
                        )
                        nc.register_instruction(nop)
                        b.instructions.insert(i, nop)
                        i += 1
                i += 1
    return nc


_NC_CACHE = {}


def _get_program(b_core):
    if b_core not in _NC_CACHE:
        _NC_CACHE[b_core] = build_program(b_core)
    return _NC_CACHE[b_core]


def _host_consts(Wq, bq, Wk, bk, Wv, bv):
    """Build pmat [128, PMAT_COLS]: identity, block-diag projection mats
    (transposed-layout), bias columns."""
    gidx = np.empty(NQ, dtype=np.int64)
    for g, (s, e, n) in enumerate(GROUPS):
        gidx[s // KV : e // KV] = g

    def mk_blockdiag(W, tok_lo, tok_hi):
        n = (tok_hi - tok_lo) * KV
        M = np.zeros((n, n), dtype=np.float32)
        for i in range(tok_lo, tok_hi):
            blk = W[gidx[i]]  # [e, d]
            r = (i - tok_lo) * KV
            # lhsT[(n,d'), (i,e)] = W[g(i)][e, d']  -> block at [r:r+9, r:r+9] = W.T
            M[r : r + KV, r : r + KV] = blk.T
        return M

    def mk_bias(b_, tok_lo, tok_hi):
        return np.concatenate([b_[gidx[i]] for i in range(tok_lo, tok_hi)]).astype(
            np.float32
        )

    pmat = np.zeros((P, PMAT_COLS), dtype=np.float32)
    pmat[:, 0:128] = np.eye(P, dtype=np.float32)
    o = 128
    for W in (Wq, Wk, Wv):
        pmat[0:NA, o : o + NA] = mk_blockdiag(np.asarray(W, np.float32), 0, 14)
        o += NA
    for W in (Wq, Wk, Wv):
        pmat[0:NB, o : o + NB] = mk_blockdiag(np.asarray(W, np.float32), 14, 25)
        o += NB
    for b_ in (bq, bk, bv):
        pmat[0:NA, o] = mk_bias(np.asarray(b_, np.float32), 0, 14); o += 1
        pmat[0:NB, o] = mk_bias(np.asarray(b_, np.float32), 14, 25); o += 1
    assert o == PMAT_COLS
    return pmat


def kernel(x, mask, Wq, bq, Wk, bk, Wv, bv, gamma, beta):
    x = np.ascontiguousarray(np.asarray(x, dtype=np.float32))
    B = x.shape[0]
    b_core = B // N_CORES
    pmat = _host_consts(Wq, bq, Wk, bk, Wv, bv)
    cst = np.concatenate([
        np.asarray(mask, dtype=np.float32).reshape(-1),
        np.asarray(gamma, dtype=np.float32).reshape(-1),
        np.asarray(beta, dtype=np.float32).reshape(-1),
    ]).astype(np.float32)
    assert cst.shape[0] == CST_LEN

    nc = _get_program(b_core)
    shards = x.reshape(N_CORES, b_core, D)
    in_maps = []
    for c in range(N_CORES):
        in_maps.append({
            "x": np.ascontiguousarray(shards[c]),
            "cst": cst,
            "pmat": pmat,
        })
    res = run_bass_kernel_spmd(nc, in_maps, core_ids=list(range(N_CORES)))
    outs = [res.results[c]["out"] for c in range(N_CORES)]
    full = np.concatenate(outs, axis=0).reshape(B, NQ, KV)
    return full.astype(np.float32)



# revision 2
# speedup vs baseline: 2.7021x; 2.7021x over previous
"""Trainium2 Bass kernel v5 for nn_AttentionSubModule: PE-centric batched
tiny attention.

Per core: 16384 items = 128 tiles of 128 items; tile = 32 groups x 4 items
(b = 128t + 4g + m). Host pre-permutes x (pure layout staging):
  X~[t, 9m+d', G*j+g...] -> xt[t, 36, (g,j)]: xt[t, 9m+d', 25g+j] = x[b, 9j+d']
  XR[t, 25m+i, 9g+e]   = x[b, 9i+e]
and post-permutes the bf16 output back to [B, 25, 9] f32.

Device pipeline per tile:
 - Projections on PE per token-group h: stationary block-diag-over-m W_h
   [36,36] bf16, moving = xt cols j in h. Evac + per-partition bias ->
   Mq/Mk [36, (j,g)] bf16, MvT [36=(m,e), (j,g)] bf16.
 - v re-layout via DRAM round trip, supertile-batched: MvT -> dmv ->
   Mv [(m,j), (r,e,g)] bf16, ones plane at e=9 (memset) gives Z for free.
 - scores: 128 per-item PE matmuls  lhsT=Mk[9m:9m+9, (j,g=g)] [9,25],
   rhs=Mq[...] [9,25] -> sT psum [(m,j), (g,i)].
 - exp on ACT, bias AP = -8 - 1e9*mask[j] (mask + range shift folded).
 - attn@v: 128 per-item PE matmuls lhsT=ex[(m,j), (g-block i)] [25,25],
   rhs=Mv slice [25,10] -> u psum [(m,i), (g, e-aug)], Z at e=9.
 - residual + LayerNorm in [(m,i), (g,e)] layout on DVE/ACT/Pool
   (LN scale-invariance absorbs the softmax normalizer, as usual).
"""

import numpy as np
from contextlib import ExitStack

import concourse.bass as bass
import concourse.tile as tile
from concourse import mybir
from concourse.bass_utils import run_bass_kernel_spmd

KV = 9
NQ = 25
D = NQ * KV
GROUPS = [(0, 27, 3), (27, 117, 10), (117, 207, 10), (207, 225, 2)]
TOK_H = [(0, 3), (3, 13), (13, 23), (23, 25)]
TOKH_OF = {0: 0, 3: 1, 13: 2, 16: 2, 23: 3}
N_CORES = 8
M4 = 4
G = 32
EPS = 1e-5
F32 = mybir.dt.float32
BF16 = mybir.dt.float16
R_SUP = 8

PM_COLS = 4 * (128 + 128 + 36)
FB_COLS = 12


def build_program_v5(T, gb_generic=False, probe=4):
    assert T % R_SUP == 0
    ST = T // R_SUP
    nc = bass.Bass("TRN2", target_bir_lowering=False)

    xt_d = nc.dram_tensor("xt", [T, 37, G * NQ], BF16, kind="ExternalInput")
    xr_d = nc.dram_tensor("xr", [T, 128, G * KV], BF16, kind="ExternalInput")
    pm_d = nc.dram_tensor("pm", [37, PM_COLS], BF16, kind="ExternalInput")
    eb_d = nc.dram_tensor("eb", [128, 1], F32, kind="ExternalInput")
    gb_d = nc.dram_tensor("gb", [128, 2 * KV], BF16, kind="ExternalInput")
    o_d = nc.dram_tensor("o", [T, 128, G * KV], BF16, kind="ExternalOutput")

    AF = mybir.ActivationFunctionType

    with tile.TileContext(nc) as tc, ExitStack() as ctx:
        consts = ctx.enter_context(tc.tile_pool(name="consts", bufs=1))
        sup = ctx.enter_context(tc.tile_pool(name="sup", bufs=2))
        dram = ctx.enter_context(tc.tile_pool(name="dram", bufs=2, space="DRAM"))
        proj = ctx.enter_context(tc.tile_pool(name="proj", bufs=2))
        expp = ctx.enter_context(tc.tile_pool(name="exsb", bufs=2 * R_SUP + 2))
        lnp = ctx.enter_context(tc.tile_pool(name="lnp", bufs=3))
        pproj = ctx.enter_context(tc.tile_pool(name="pproj", bufs=2, space="PSUM"))
        psc = ctx.enter_context(tc.tile_pool(name="psc", bufs=2, space="PSUM"))
        pu = ctx.enter_context(tc.tile_pool(name="pu", bufs=2, space="PSUM"))

        pm_t = consts.tile([37, PM_COLS], BF16)
        nc.sync.dma_start(out=pm_t, in_=pm_d[:, :])
        eb_t = consts.tile([128, 1], F32)
        nc.sync.dma_start(out=eb_t, in_=eb_d[:, :])
        gb_t = consts.tile([128, 2 * KV], BF16)
        nc.sync.dma_start(out=gb_t, in_=gb_d[:, :])

        Wmat = {}
        for h in range(4):
            c = (128 + 128 + 36) * h
            Wmat["q", h] = pm_t[:, c : c + 128]
            Wmat["k", h] = pm_t[:, c + 128 : c + 256]
            Wmat["v", h] = pm_t[:, c + 256 : c + 292]

        inv3 = float(1.0 / np.sqrt(KV))

        for s in range(ST):
            xts = sup.tile([37, R_SUP * G * NQ], BF16, tag="xts")
            nc.sync.dma_start(
                out=xts[:].rearrange("p (r c) -> p r c", r=R_SUP),
                in_=xt_d[s * R_SUP : (s + 1) * R_SUP, :, :].transpose([1, 0, 2]),
            )
            xrs = sup.tile([128, R_SUP * G * KV], BF16, tag="xrs")
            nc.sync.dma_start(
                out=xrs[:].rearrange("p (r c) -> p r c", r=R_SUP),
                in_=xr_d[s * R_SUP : (s + 1) * R_SUP, :, :].transpose([1, 0, 2]),
            )
            mvts = sup.tile([36, R_SUP * NQ * G], BF16, tag="mvts")
            os_t = sup.tile([128, R_SUP * G * KV], BF16, tag="os")

            exs = []
            # ---------- pass 1: projections, scores, exp ----------
            for r in range(R_SUP):
                xv = xts[:, r * G * NQ : (r + 1) * G * NQ].rearrange(
                    "p (g j) -> p g j", g=G)

                mq_t = proj.tile([128, NQ * G], BF16, tag="mq")
                mk_t = proj.tile([128, NQ * G], BF16, tag="mk")
                mvt = mvts[:, r * NQ * G : (r + 1) * NQ * G]
                # q/k/v sequentially through one 2-bank psum tag.
                # q/k psum layout: col = 25*g + j for g<16, 512 + 25*(g-16) + j
                # v psum layout: col = 32*j + g (contiguous across banks at j=16)
                for nm in ("q", "k", "v"):
                    rows = 36 if nm == "v" else 128
                    pp = pproj.tile([128, 1024], F32, tag="pp")
                    if nm == "v":
                        # split by j at the bank boundary (j=16)
                        for j0, j1 in ((0, 3), (3, 13), (13, 16),
                                       (16, 23), (23, 25)):
                            w = j1 - j0
                            rhs = xts[:, r * G * NQ : (r + 1) * G * NQ].rearrange(
                                "p (g j) -> p g j", g=G)[:, :, j0:j1]
                            dst = bass.AP(
                                tensor=pp[:].tensor,
                                offset=pp[:].offset + 32 * j0,
                                ap=[[pp[:].ap[0][0], rows], [1, G], [G, w]],
                            )
                            nc.tensor.matmul(dst, Wmat[nm, TOKH_OF[j0]], rhs,
                                             start=True, stop=True)
                        nc.scalar.copy(mvt, pp[0:36, 0:800])
                    else:
                        for gh in range(2):
                            for h in range(4):
                                j0, j1 = TOK_H[h]
                                w = j1 - j0
                                rhs = xts[:, r * G * NQ : (r + 1) * G * NQ
                                          ].rearrange("p (g j) -> p g j", g=G)[
                                    :, 16 * gh : 16 * gh + 16, j0:j1]
                                dst = bass.AP(
                                    tensor=pp[:].tensor,
                                    offset=pp[:].offset + 512 * gh + j0,
                                    ap=[[pp[:].ap[0][0], rows], [NQ, 16], [1, w]],
                                )
                                nc.tensor.matmul(dst, Wmat[nm, h], rhs,
                                                 start=True, stop=True)
                        mdst = (mq_t if nm == "q" else mk_t)[:].rearrange(
                            "p (gh c) -> p gh c", gh=2)
                        msrc = bass.AP(
                            tensor=pp[:].tensor, offset=pp[:].offset,
                            ap=[[pp[:].ap[0][0], 128], [512, 2], [1, 400]],
                        )
                        if nm == "q":
                            nc.scalar.copy(mdst, msrc)
                        else:
                            nc.vector.tensor_copy(mdst, msrc)

                if probe <= 1:
                    nc.vector.tensor_copy(
                        os_t[:, r * G * KV : (r + 1) * G * KV],
                        mq_t[:, 0 : G * KV])
                    continue
                # scores: one 2-bank psum, g<16 at cols 25g+j, g>=16 at
                # 512 + 25(g-16) + j
                sc = psc.tile([128, 1024], F32, tag="sc", bufs=1)
                mk3 = mk_t[:].rearrange("p (g j) -> p g j", g=G)
                mq3 = mq_t[:].rearrange("p (g j) -> p g j", g=G)
                for g in range(G):
                    c0 = 25 * g if g < 16 else 512 + 25 * (g - 16)
                    for m in range(M4):
                        nc.tensor.matmul(
                            sc[32 * m : 32 * m + 25, c0 : c0 + 25],
                            mk3[32 * m : 32 * m + 9, g, :],
                            mq3[32 * m : 32 * m + 9, g, :],
                            start=True, stop=True,
                            tile_position=(32 * m, 32 * m))

                ex_t = expp.tile([128, G * NQ], BF16, tag="ex")
                sc_v = bass.AP(
                    tensor=sc[:].tensor, offset=sc[:].offset,
                    ap=[[sc[:].ap[0][0], 128], [512, 2], [1, 400]],
                )
                nc.scalar.activation(
                    ex_t[:].rearrange("p (h c) -> p h c", h=2), sc_v, AF.Exp,
                    bias=eb_t[:], scale=inv3)
                exs.append(ex_t)
                if probe <= 2:
                    nc.vector.tensor_copy(
                        os_t[:, r * G * KV : (r + 1) * G * KV],
                        ex_t[:, 0 : G * KV])

            if probe <= 2:
                nc.sync.dma_start(
                    out=o_d[s * R_SUP : (s + 1) * R_SUP, :, :].transpose([1, 0, 2]),
                    in_=os_t[:].rearrange("p (r c) -> p r c", r=R_SUP),
                )
                continue
            # ---------- v round trip (supertile) ----------
            # dmv element layout: off = 28800 r + 1152 j + 288 m + 32 e + g
            dmv = dram.tile([36, R_SUP * NQ * G], BF16, tag="dmv")
            dmv_ap = dmv[:]
            dump_dst = bass.AP(
                tensor=dmv_ap.tensor, offset=dmv_ap.offset,
                ap=[[32, 36], [1152, R_SUP * NQ], [1, G]],
            )
            nc.scalar.dma_start(out=dump_dst, in_=mvts[:].rearrange(
                "p (rj g) -> p rj g", g=G))
            mv_s = sup.tile([128, R_SUP * 10 * G], BF16, tag="mvs")
            mv4 = mv_s[:].rearrange("p (r e g) -> p r e g", r=R_SUP, e=10)
            nc.vector.memset(mv4[:, :, 9, :], 1.0)
            for m in range(M4):
                dstv = mv4[32 * m : 32 * m + 25, :, 0:9, :]  # [25, R, 9, G]
                srcv = bass.AP(
                    tensor=dmv_ap.tensor,
                    offset=dmv_ap.offset + 288 * m,
                    ap=[[1152, NQ], [28800, R_SUP], [1, KV * G]],
                )
                nc.sync.dma_start(out=dstv, in_=srcv)

            # ---------- pass 2: attn@v + residual + LN ----------
            for r in range(R_SUP):
                ex_t = exs[r]
                u_ps = pu.tile([128, 320], F32, tag="u")
                mv_r = mv4[:, r, :, :]  # [100, 10, G]
                for g in range(G):
                    e0 = 25 * g if g < 16 else 400 + 25 * (g - 16)
                    for m in range(M4):
                        nc.tensor.matmul(
                            u_ps[32 * m : 32 * m + 25, 10 * g : 10 * g + 10],
                            ex_t[32 * m : 32 * m + 25, e0 : e0 + 25],
                            mv_r[32 * m : 32 * m + 25, :, g],
                            start=True, stop=True,
                            tile_position=(32 * m, 32 * m))

                u_s = lnp.tile([128, 320], F32, tag="us")
                nc.scalar.copy(u_s[:], u_ps[:])
                if probe <= 3:
                    nc.vector.tensor_copy(
                        os_t[:, r * G * KV : (r + 1) * G * KV],
                        u_s[:, 0 : G * KV])
                    continue
                u3 = u_s[:].rearrange("p (g e) -> p g e", g=G)
                uvec = u3[:, :, 0:9]
                z_ap = u3[:, :, 9:10]
                xr_r = xrs[:, r * G * KV : (r + 1) * G * KV].rearrange(
                    "p (g e) -> p g e", g=G)

                zx = lnp.tile([128, G, KV], F32, tag="zx")
                nc.gpsimd.tensor_mul(zx[:], xr_r,
                                     z_ap.broadcast_to((128, G, KV)))
                u2 = lnp.tile([128, G, KV], F32, tag="u2")
                nc.gpsimd.tensor_add(u2[:], uvec, zx[:])
                s_t = lnp.tile([128, G], F32, tag="s")
                nc.vector.tensor_reduce(
                    s_t[:], u2[:], axis=mybir.AxisListType.X,
                    op=mybir.AluOpType.add)
                mu = lnp.tile([128, G], F32, tag="mu")
                nc.gpsimd.tensor_scalar_mul(mu[:], s_t[:], 1.0 / KV)
                cen = lnp.tile([128, G, KV], F32, tag="cen")
                nc.vector.tensor_sub(
                    cen[:], u2[:],
                    mu[:].unsqueeze(2).broadcast_to((128, G, KV)))
                sq = lnp.tile([128, G, KV], F32, tag="sq")
                nc.gpsimd.tensor_mul(sq[:], cen[:], cen[:])
                vs = lnp.tile([128, G], F32, tag="vs")
                nc.vector.tensor_reduce(
                    vs[:], sq[:], axis=mybir.AxisListType.X,
                    op=mybir.AluOpType.add)
                zsq = lnp.tile([128, G], F32, tag="zsq")
                nc.gpsimd.tensor_mul(
                    zsq[:], z_ap.rearrange("p a b -> p (a b)"),
                    z_ap.rearrange("p a b -> p (a b)"))
                vs2 = lnp.tile([128, G], F32, tag="vs2")
                nc.vector.scalar_tensor_tensor(
                    vs2[:], zsq[:], float(KV * EPS), vs[:],
                    op0=mybir.AluOpType.mult, op1=mybir.AluOpType.add)
                sd = lnp.tile([128, G], F32, tag="sd")
                nc.scalar.activation(sd[:], vs2[:], AF.Sqrt,
                                     bias=0.0, scale=1.0 / KV)
                rstd = lnp.tile([128, G], F32, tag="rstd")
                nc.vector.reciprocal(rstd[:], sd[:])

                o_r = os_t[:, r * G * KV : (r + 1) * G * KV].rearrange(
                    "p (g e) -> p g e", g=G)
                if gb_generic:
                    o1 = lnp.tile([128, G, KV], F32, tag="o1")
                    nc.vector.tensor_mul(
                        o1[:], cen[:],
                        rstd[:].unsqueeze(2).broadcast_to((128, G, KV)))
                    gam = bass.AP(tensor=gb_t[:].tensor, offset=gb_t[:].offset,
                                  ap=[[0, 128], [0, G], [1, KV]])
                    bet = bass.AP(tensor=gb_t[:].tensor,
                                  offset=gb_t[:].offset + KV,
                                  ap=[[0, 128], [0, G], [1, KV]])
                    o2 = lnp.tile([128, G, KV], F32, tag="o2")
                    nc.vector.tensor_mul(o2[:], o1[:], gam)
                    nc.vector.tensor_add(o_r, o2[:], bet)
                else:
                    nc.vector.tensor_mul(
                        o_r, cen[:],
                        rstd[:].unsqueeze(2).broadcast_to((128, G, KV)))

            nc.sync.dma_start(
                out=o_d[s * R_SUP : (s + 1) * R_SUP, :, :].transpose([1, 0, 2]),
                in_=os_t[:].rearrange("p (r c) -> p r c", r=R_SUP),
            )

    _split_multi_waits(nc)
    return nc


def _split_multi_waits(nc):
    for f in nc.m.functions:
        for b in f.blocks:
            i = 0
            while i < len(b.instructions):
                inst = b.instructions[i]
                si = getattr(inst, "sync_info", None)
                if si is not None and si.on_wait and len(si.on_wait) > 1:
                    extra = si.on_wait[:-1]
                    si.on_wait = si.on_wait[-1:]
                    for w in extra:
                        nop = mybir.InstNoOp(
                            name=nc.get_next_instruction_name(),
                            engine=inst.engine, ins=[], outs=[],
                            sync_info=mybir.SyncInfo(on_wait=[w], on_update=[]),
                        )
                        nc.register_instruction(nop)
                        b.instructions.insert(i, nop)
                        i += 1
                i += 1
    return nc


# ---------------- host side ----------------

def _group_mats(W, b_):
    """Expand per-group (4) mats to per-token-group-h lists."""
    W = np.asarray(W, np.float32)
    b_ = np.asarray(b_, np.float32)
    return [W[h] for h in range(4)], [b_[h] for h in range(4)]


def _host_consts(Wq, bq, Wk, bk, Wv, bv, mask, gamma, beta):
    pm = np.zeros((37, PM_COLS), np.float32)
    for h in range(4):
        c = (128 + 128 + 36) * h
        Wqh = np.asarray(Wq, np.float32)[h]
        Wkh = np.asarray(Wk, np.float32)[h]
        Wvh = np.asarray(Wv, np.float32)[h]
        for m in range(M4):
            pm[9 * m : 9 * m + 9, c + 32 * m : c + 32 * m + 9] = Wqh.T
            pm[9 * m : 9 * m + 9, c + 128 + 32 * m : c + 128 + 32 * m + 9] = Wkh.T
            pm[9 * m : 9 * m + 9, c + 256 + 9 * m : c + 256 + 9 * m + 9] = Wvh.T
            pm[36, c + 32 * m : c + 32 * m + 9] = np.asarray(bq, np.float32)[h]
            pm[36, c + 128 + 32 * m : c + 128 + 32 * m + 9] = np.asarray(
                bk, np.float32)[h]
            pm[36, c + 256 + 9 * m : c + 256 + 9 * m + 9] = np.asarray(
                bv, np.float32)[h]

    eb = np.full((128, 1), -8.0, np.float32)
    mk = np.asarray(mask, np.float32)
    for m in range(M4):
        for j in range(NQ):
            eb[32 * m + j, 0] = -8.0 - 1e9 * float(mk[j])
    gb = np.concatenate([
        np.broadcast_to(np.asarray(gamma, np.float32), (128, KV)),
        np.broadcast_to(np.asarray(beta, np.float32), (128, KV)),
    ], axis=1)
    return pm, eb, np.ascontiguousarray(gb)


def host_layouts(x, T):
    xt4 = x.reshape(T, G, M4, NQ, KV)  # [t, g, m, j, d]
    xtil = np.ones((T, 37, G * NQ), np.float32)
    xtil[:, 0:36] = xt4.transpose(0, 2, 4, 1, 3).reshape(T, 36, G * NQ)
    xr = np.zeros((T, 128, G * KV), np.float16)
    xr4 = xt4.transpose(0, 2, 3, 1, 4).reshape(T, M4, NQ, G * KV)
    for m in range(M4):
        xr[:, 32 * m : 32 * m + 25, :] = xr4[:, m].astype(np.float16)
    return xtil.astype(np.float16), xr


def unpermute_out(o, T):
    o6 = o.reshape(T, 4, 32, G, KV)[:, :, :NQ]  # [t, m, i, g, e]
    o5 = o6.transpose(0, 3, 1, 2, 4)  # [t, g, m, i, e]
    return np.ascontiguousarray(o5).reshape(T * 128, NQ, KV).astype(np.float32)


_NC_CACHE = {}


def _get_program(T, gb_generic):
    key = (T, gb_generic)
    if key not in _NC_CACHE:
        _NC_CACHE[key] = build_program_v5(T, gb_generic)
    return _NC_CACHE[key]


def kernel(x, mask, Wq, bq, Wk, bk, Wv, bv, gamma, beta):
    x = np.ascontiguousarray(np.asarray(x, dtype=np.float32))
    B = x.shape[0]
    b_core = B // N_CORES
    T = b_core // 128
    gb_generic = not (np.all(np.asarray(gamma) == 1.0)
                      and np.all(np.asarray(beta) == 0.0))
    pm, eb, gb = _host_consts(Wq, bq, Wk, bk, Wv, bv, mask, gamma, beta)
    nc = _get_program(T, gb_generic)

    shards = x.reshape(N_CORES, b_core, D)
    in_maps = []
    for c in range(N_CORES):
        xtil, xr = host_layouts(shards[c], T)
        in_maps.append({
            "xt": xtil, "xr": xr,
            "pm": pm.astype(np.float16), "eb": eb,
            "gb": gb.astype(np.float16),
        })
    res = run_bass_kernel_spmd(nc, in_maps, core_ids=list(range(N_CORES)))
    outs = [unpermute_out(np.asarray(res.results[c]["o"], np.float32), T)
            for c in range(N_CORES)]
    return np.concatenate(outs, axis=0)


# revision 3
# speedup vs baseline: 3.2039x; 1.1857x over previous
"""Trainium2 Bass kernel v5 for nn_AttentionSubModule: PE-centric batched
tiny attention.

Per core: 16384 items = 128 tiles of 128 items; tile = 32 groups x 4 items
(b = 128t + 4g + m). Host pre-permutes x (pure layout staging):
  X~[t, 9m+d', G*j+g...] -> xt[t, 36, (g,j)]: xt[t, 9m+d', 25g+j] = x[b, 9j+d']
  XR[t, 25m+i, 9g+e]   = x[b, 9i+e]
and post-permutes the bf16 output back to [B, 25, 9] f32.

Device pipeline per tile:
 - Projections on PE per token-group h: stationary block-diag-over-m W_h
   [36,36] bf16, moving = xt cols j in h. Evac + per-partition bias ->
   Mq/Mk [36, (j,g)] bf16, MvT [36=(m,e), (j,g)] bf16.
 - v re-layout via DRAM round trip, supertile-batched: MvT -> dmv ->
   Mv [(m,j), (r,e,g)] bf16, ones plane at e=9 (memset) gives Z for free.
 - scores: 128 per-item PE matmuls  lhsT=Mk[9m:9m+9, (j,g=g)] [9,25],
   rhs=Mq[...] [9,25] -> sT psum [(m,j), (g,i)].
 - exp on ACT, bias AP = -8 - 1e9*mask[j] (mask + range shift folded).
 - attn@v: 128 per-item PE matmuls lhsT=ex[(m,j), (g-block i)] [25,25],
   rhs=Mv slice [25,10] -> u psum [(m,i), (g, e-aug)], Z at e=9.
 - residual + LayerNorm in [(m,i), (g,e)] layout on DVE/ACT/Pool
   (LN scale-invariance absorbs the softmax normalizer, as usual).
"""

import numpy as np
from contextlib import ExitStack

import concourse.bass as bass
import concourse.tile as tile
from concourse import mybir
from concourse.bass_utils import run_bass_kernel_spmd

KV = 9
NQ = 25
D = NQ * KV
GROUPS = [(0, 27, 3), (27, 117, 10), (117, 207, 10), (207, 225, 2)]
TOK_H = [(0, 3), (3, 13), (13, 23), (23, 25)]
TOKH_OF = {0: 0, 3: 1, 13: 2, 16: 2, 23: 3}
N_CORES = 8
M4 = 4
G = 32
EPS = 1e-5
F32 = mybir.dt.float32
BF16 = mybir.dt.float16
R_SUP = 8

PM_COLS = 4 * (128 + 128 + 36)
FB_COLS = 12


def build_program_v5(T, gb_generic=False, probe=4):
    assert T % R_SUP == 0
    ST = T // R_SUP
    nc = bass.Bass("TRN2", target_bir_lowering=False)

    xt_d = nc.dram_tensor("xt", [T, 37, G * NQ], BF16, kind="ExternalInput")
    xr_d = nc.dram_tensor("xr", [T, 128, G * KV], BF16, kind="ExternalInput")
    pm_d = nc.dram_tensor("pm", [37, PM_COLS], BF16, kind="ExternalInput")
    eb_d = nc.dram_tensor("eb", [128, 1], F32, kind="ExternalInput")
    gb_d = nc.dram_tensor("gb", [128, 2 * KV], BF16, kind="ExternalInput")
    o_d = nc.dram_tensor("o", [T, 128, G * KV], BF16, kind="ExternalOutput")

    AF = mybir.ActivationFunctionType

    with tile.TileContext(nc) as tc, ExitStack() as ctx:
        consts = ctx.enter_context(tc.tile_pool(name="consts", bufs=1))
        sup = ctx.enter_context(tc.tile_pool(name="sup", bufs=2))
        dram = ctx.enter_context(tc.tile_pool(name="dram", bufs=2, space="DRAM"))
        proj = ctx.enter_context(tc.tile_pool(name="proj", bufs=3))
        expp = ctx.enter_context(tc.tile_pool(name="exsb", bufs=2 * R_SUP + 2))
        lnp = ctx.enter_context(tc.tile_pool(name="lnp", bufs=3))
        pproj = ctx.enter_context(tc.tile_pool(name="pproj", bufs=2, space="PSUM"))
        psc = ctx.enter_context(tc.tile_pool(name="psc", bufs=2, space="PSUM"))
        pu = ctx.enter_context(tc.tile_pool(name="pu", bufs=2, space="PSUM"))

        pm_t = consts.tile([37, PM_COLS], BF16)
        nc.sync.dma_start(out=pm_t, in_=pm_d[:, :])
        eb_t = consts.tile([128, 1], F32)
        nc.sync.dma_start(out=eb_t, in_=eb_d[:, :])
        gb_t = consts.tile([128, 2 * KV], BF16)
        nc.sync.dma_start(out=gb_t, in_=gb_d[:, :])

        Wmat = {}
        for h in range(4):
            c = (128 + 128 + 36) * h
            Wmat["q", h] = pm_t[:, c : c + 128]
            Wmat["k", h] = pm_t[:, c + 128 : c + 256]
            Wmat["v", h] = pm_t[:, c + 256 : c + 292]

        inv3 = float(1.0 / np.sqrt(KV))

        for s in range(ST):
            xts = sup.tile([37, R_SUP * G * NQ], BF16, tag="xts")
            nc.sync.dma_start(
                out=xts[:].rearrange("p (r c) -> p r c", r=R_SUP),
                in_=xt_d[s * R_SUP : (s + 1) * R_SUP, :, :].transpose([1, 0, 2]),
            )
            xrs = sup.tile([128, R_SUP * G * KV], BF16, tag="xrs")
            nc.sync.dma_start(
                out=xrs[:].rearrange("p (r c) -> p r c", r=R_SUP),
                in_=xr_d[s * R_SUP : (s + 1) * R_SUP, :, :].transpose([1, 0, 2]),
            )
            mvts = sup.tile([36, R_SUP * NQ * G], BF16, tag="mvts")
            os_t = sup.tile([128, R_SUP * G * KV], BF16, tag="os")

            exs = []
            # ---------- pass 1: projections, scores, exp ----------
            for r in range(R_SUP):
                xv = xts[:, r * G * NQ : (r + 1) * G * NQ].rearrange(
                    "p (g j) -> p g j", g=G)

                mq_t = proj.tile([128, NQ * G], BF16, tag="mq")
                mk_t = proj.tile([128, NQ * G], BF16, tag="mk")
                mvt = mvts[:, r * NQ * G : (r + 1) * NQ * G]
                # q/k/v sequentially through one 2-bank psum tag.
                # q/k psum layout: col = 25*g + j for g<16, 512 + 25*(g-16) + j
                # v psum layout: col = 32*j + g (contiguous across banks at j=16)
                for nm in ("q", "k", "v"):
                    rows = 36 if nm == "v" else 128
                    pp = pproj.tile([128, 1024], F32, tag="pp")
                    if nm == "v":
                        # split by j at the bank boundary (j=16)
                        for j0, j1 in ((0, 3), (3, 13), (13, 16),
                                       (16, 23), (23, 25)):
                            w = j1 - j0
                            rhs = xts[:, r * G * NQ : (r + 1) * G * NQ].rearrange(
                                "p (g j) -> p g j", g=G)[:, :, j0:j1]
                            dst = bass.AP(
                                tensor=pp[:].tensor,
                                offset=pp[:].offset + 32 * j0,
                                ap=[[pp[:].ap[0][0], rows], [1, G], [G, w]],
                            )
                            nc.tensor.matmul(dst, Wmat[nm, TOKH_OF[j0]], rhs,
                                             start=True, stop=True)
                        nc.scalar.copy(mvt, pp[0:36, 0:800])
                    else:
                        for gh in range(2):
                            for h in range(4):
                                j0, j1 = TOK_H[h]
                                w = j1 - j0
                                rhs = xts[:, r * G * NQ : (r + 1) * G * NQ
                                          ].rearrange("p (g j) -> p g j", g=G)[
                                    :, 16 * gh : 16 * gh + 16, j0:j1]
                                dst = bass.AP(
                                    tensor=pp[:].tensor,
                                    offset=pp[:].offset + 512 * gh + j0,
                                    ap=[[pp[:].ap[0][0], rows], [NQ, 16], [1, w]],
                                )
                                nc.tensor.matmul(dst, Wmat[nm, h], rhs,
                                                 start=True, stop=True)
                        mdst = (mq_t if nm == "q" else mk_t)[:].rearrange(
                            "p (gh c) -> p gh c", gh=2)
                        msrc = bass.AP(
                            tensor=pp[:].tensor, offset=pp[:].offset,
                            ap=[[pp[:].ap[0][0], 128], [512, 2], [1, 400]],
                        )
                        if nm == "q":
                            nc.scalar.copy(mdst, msrc)
                        else:
                            nc.vector.tensor_copy(mdst, msrc)

                if probe <= 1:
                    nc.vector.tensor_copy(
                        os_t[:, r * G * KV : (r + 1) * G * KV],
                        mq_t[:, 0 : G * KV])
                    continue
                # scores: one 2-bank psum, g<16 at cols 25g+j, g>=16 at
                # 512 + 25(g-16) + j
                sc = psc.tile([128, 1024], F32, tag="sc", bufs=1)
                mk3 = mk_t[:].rearrange("p (g j) -> p g j", g=G)
                mq3 = mq_t[:].rearrange("p (g j) -> p g j", g=G)
                for g in range(G):
                    c0 = 25 * g if g < 16 else 512 + 25 * (g - 16)
                    for m in range(M4):
                        nc.tensor.matmul(
                            sc[32 * m : 32 * m + 25, c0 : c0 + 25],
                            mk3[32 * m : 32 * m + 9, g, :],
                            mq3[32 * m : 32 * m + 9, g, :],
                            start=True, stop=True,
                            tile_position=(32 * m, 32 * m))

                ex_t = expp.tile([128, G * NQ], BF16, tag="ex")
                sc_v = bass.AP(
                    tensor=sc[:].tensor, offset=sc[:].offset,
                    ap=[[sc[:].ap[0][0], 128], [512, 2], [1, 400]],
                )
                nc.scalar.activation(
                    ex_t[:].rearrange("p (h c) -> p h c", h=2), sc_v, AF.Exp,
                    bias=eb_t[:], scale=inv3)
                exs.append(ex_t)
                if probe <= 2:
                    nc.vector.tensor_copy(
                        os_t[:, r * G * KV : (r + 1) * G * KV],
                        ex_t[:, 0 : G * KV])

            if probe <= 2:
                nc.sync.dma_start(
                    out=o_d[s * R_SUP : (s + 1) * R_SUP, :, :].transpose([1, 0, 2]),
                    in_=os_t[:].rearrange("p (r c) -> p r c", r=R_SUP),
                )
                continue
            RH = R_SUP // 4
            for hf in range(4):
                # ---------- v round trip (half supertile) ----------
                # dmv element layout: off = 28800 rh + 1152 j + 288 m + 32 e + g
                dmv = dram.tile([36, RH * NQ * G], BF16, tag=f"dmv{hf}")
                dmv_ap = dmv[:]
                dump_dst = bass.AP(
                    tensor=dmv_ap.tensor, offset=dmv_ap.offset,
                    ap=[[32, 36], [1152, RH * NQ], [1, G]],
                )
                mvh = mvts[:, hf * RH * NQ * G : (hf + 1) * RH * NQ * G]
                nc.sync.dma_start(out=dump_dst, in_=mvh.rearrange(
                    "p (rj g) -> p rj g", g=G))
                mv_s = sup.tile([128, RH * 10 * G], BF16, tag=f"mvs{hf}")
                mv4 = mv_s[:].rearrange("p (r e g) -> p r e g", r=RH, e=10)
                nc.vector.memset(mv4[:, :, 9, :], 1.0)
                for m in range(M4):
                    dstv = mv4[32 * m : 32 * m + 25, :, 0:9, :]
                    srcv = bass.AP(
                        tensor=dmv_ap.tensor,
                        offset=dmv_ap.offset + 288 * m,
                        ap=[[1152, NQ], [28800, RH], [1, KV * G]],
                    )
                    nc.sync.dma_start(out=dstv, in_=srcv)

                # ---------- pass 2 on this half (tile pairs) ----------
                for r2 in range(RH // 2):
                    u_s = lnp.tile([128, 2, 320], F32, tag="us")
                    for half in range(2):
                        rh = 2 * r2 + half
                        r = hf * RH + rh
                        ex_t = exs[r]
                        u_ps = pu.tile([128, 320], F32, tag="u")
                        mv_r = mv4[:, rh, :, :]
                        for g in range(G):
                            e0 = 25 * g if g < 16 else 400 + 25 * (g - 16)
                            for m in range(M4):
                                nc.tensor.matmul(
                                    u_ps[32 * m : 32 * m + 25,
                                         10 * g : 10 * g + 10],
                                    ex_t[32 * m : 32 * m + 25, e0 : e0 + 25],
                                    mv_r[32 * m : 32 * m + 25, :, g],
                                    start=True, stop=True,
                                    tile_position=(32 * m, 32 * m))
                        nc.scalar.copy(u_s[:, half, :], u_ps[:])

                    r = hf * RH + 2 * r2
                    u3 = u_s[:].rearrange("p h (g e) -> p (h g) e", g=G)
                    uvec = u3[:, :, 0:9]
                    z_ap = u3[:, :, 9:10]
                    G2 = 2 * G
                    xr_r = xrs[:, r * G * KV : (r + 2) * G * KV].rearrange(
                        "p (g e) -> p g e", g=G2)

                    zx = lnp.tile([128, G2, KV], F32, tag="zx")
                    nc.gpsimd.tensor_mul(zx[:], xr_r,
                                         z_ap.broadcast_to((128, G2, KV)))
                    u2 = lnp.tile([128, G2, KV], F32, tag="u2")
                    nc.gpsimd.tensor_add(u2[:], uvec, zx[:])
                    s_t = lnp.tile([128, G2], F32, tag="s")
                    nc.vector.tensor_reduce(
                        s_t[:], u2[:], axis=mybir.AxisListType.X,
                        op=mybir.AluOpType.add)
                    mu = lnp.tile([128, G2], F32, tag="mu")
                    nc.gpsimd.tensor_scalar_mul(mu[:], s_t[:], 1.0 / KV)
                    cen = lnp.tile([128, G2, KV], F32, tag="cen")
                    nc.vector.tensor_sub(
                        cen[:], u2[:],
                        mu[:].unsqueeze(2).broadcast_to((128, G2, KV)))
                    sq = lnp.tile([128, G2, KV], F32, tag="sq")
                    nc.gpsimd.tensor_mul(sq[:], cen[:], cen[:])
                    vs = lnp.tile([128, G2], F32, tag="vs")
                    nc.vector.tensor_reduce(
                        vs[:], sq[:], axis=mybir.AxisListType.X,
                        op=mybir.AluOpType.add)
                    zsq = lnp.tile([128, G2], F32, tag="zsq")
                    zf = z_ap.rearrange("p a b -> p (a b)")
                    nc.gpsimd.tensor_mul(zsq[:], zf, zf)
                    vs2 = lnp.tile([128, G2], F32, tag="vs2")
                    nc.vector.scalar_tensor_tensor(
                        vs2[:], zsq[:], float(KV * EPS), vs[:],
                        op0=mybir.AluOpType.mult, op1=mybir.AluOpType.add)
                    sd = lnp.tile([128, G2], F32, tag="sd")
                    nc.scalar.activation(sd[:], vs2[:], AF.Sqrt,
                                         bias=0.0, scale=1.0 / KV)
                    rstd = lnp.tile([128, G2], F32, tag="rstd")
                    nc.vector.reciprocal(rstd[:], sd[:])

                    o_r = os_t[:, r * G * KV : (r + 2) * G * KV].rearrange(
                        "p (g e) -> p g e", g=G2)
                    if gb_generic:
                        o1 = lnp.tile([128, G2, KV], F32, tag="o1")
                        nc.vector.tensor_mul(
                            o1[:], cen[:],
                            rstd[:].unsqueeze(2).broadcast_to((128, G2, KV)))
                        gam = bass.AP(tensor=gb_t[:].tensor,
                                      offset=gb_t[:].offset,
                                      ap=[[0, 128], [0, G2], [1, KV]])
                        bet = bass.AP(tensor=gb_t[:].tensor,
                                      offset=gb_t[:].offset + KV,
                                      ap=[[0, 128], [0, G2], [1, KV]])
                        o2 = lnp.tile([128, G2, KV], F32, tag="o2")
                        nc.vector.tensor_mul(o2[:], o1[:], gam)
                        nc.vector.tensor_add(o_r, o2[:], bet)
                    else:
                        nc.vector.tensor_mul(
                            o_r, cen[:],
                            rstd[:].unsqueeze(2).broadcast_to((128, G2, KV)))

            nc.sync.dma_start(
                out=o_d[s * R_SUP : (s + 1) * R_SUP, :, :].transpose([1, 0, 2]),
                in_=os_t[:].rearrange("p (r c) -> p r c", r=R_SUP),
            )

    _split_multi_waits(nc)
    return nc


def _split_multi_waits(nc):
    for f in nc.m.functions:
        for b in f.blocks:
            i = 0
            while i < len(b.instructions):
                inst = b.instructions[i]
                si = getattr(inst, "sync_info", None)
                if si is not None and si.on_wait and len(si.on_wait) > 1:
                    extra = si.on_wait[:-1]
                    si.on_wait = si.on_wait[-1:]
                    for w in extra:
                        nop = mybir.InstNoOp(
                            name=nc.get_next_instruction_name(),
                            engine=inst.engine, ins=[], outs=[],
                            sync_info=mybir.SyncInfo(on_wait=[w], on_update=[]),
                        )
                        nc.register_instruction(nop)
                        b.instructions.insert(i, nop)
                        i += 1
                i += 1
    return nc


# ---------------- host side ----------------

def _group_mats(W, b_):
    """Expand per-group (4) mats to per-token-group-h lists."""
    W = np.asarray(W, np.float32)
    b_ = np.asarray(b_, np.float32)
    return [W[h] for h in range(4)], [b_[h] for h in range(4)]


def _host_consts(Wq, bq, Wk, bk, Wv, bv, mask, gamma, beta):
    pm = np.zeros((37, PM_COLS), np.float32)
    for h in range(4):
        c = (128 + 128 + 36) * h
        Wqh = np.asarray(Wq, np.float32)[h]
        Wkh = np.asarray(Wk, np.float32)[h]
        Wvh = np.asarray(Wv, np.float32)[h]
        for m in range(M4):
            pm[9 * m : 9 * m + 9, c + 32 * m : c + 32 * m + 9] = Wqh.T
            pm[9 * m : 9 * m + 9, c + 128 + 32 * m : c + 128 + 32 * m + 9] = Wkh.T
            pm[9 * m : 9 * m + 9, c + 256 + 9 * m : c + 256 + 9 * m + 9] = Wvh.T
            pm[36, c + 32 * m : c + 32 * m + 9] = np.asarray(bq, np.float32)[h]
            pm[36, c + 128 + 32 * m : c + 128 + 32 * m + 9] = np.asarray(
                bk, np.float32)[h]
            pm[36, c + 256 + 9 * m : c + 256 + 9 * m + 9] = np.asarray(
                bv, np.float32)[h]

    eb = np.full((128, 1), -8.0, np.float32)
    mk = np.asarray(mask, np.float32)
    for m in range(M4):
        for j in range(NQ):
            eb[32 * m + j, 0] = -8.0 - 1e9 * float(mk[j])
    gb = np.concatenate([
        np.broadcast_to(np.asarray(gamma, np.float32), (128, KV)),
        np.broadcast_to(np.asarray(beta, np.float32), (128, KV)),
    ], axis=1)
    return pm, eb, np.ascontiguousarray(gb)


def host_layouts(x, T):
    xt4 = x.reshape(T, G, M4, NQ, KV)  # [t, g, m, j, d]
    xtil = np.ones((T, 37, G * NQ), np.float32)
    xtil[:, 0:36] = xt4.transpose(0, 2, 4, 1, 3).reshape(T, 36, G * NQ)
    xr = np.zeros((T, 128, G * KV), np.float16)
    xr4 = xt4.transpose(0, 2, 3, 1, 4).reshape(T, M4, NQ, G * KV)
    for m in range(M4):
        xr[:, 32 * m : 32 * m + 25, :] = xr4[:, m].astype(np.float16)
    return xtil.astype(np.float16), xr


def unpermute_out(o, T):
    o6 = o.reshape(T, 4, 32, G, KV)[:, :, :NQ]  # [t, m, i, g, e]
    o5 = o6.transpose(0, 3, 1, 2, 4)  # [t, g, m, i, e]
    return np.ascontiguousarray(o5).reshape(T * 128, NQ, KV).astype(np.float32)


_NC_CACHE = {}


def _get_program(T, gb_generic):
    key = (T, gb_generic)
    if key not in _NC_CACHE:
        _NC_CACHE[key] = build_program_v5(T, gb_generic)
    return _NC_CACHE[key]


def kernel(x, mask, Wq, bq, Wk, bk, Wv, bv, gamma, beta):
    x = np.ascontiguousarray(np.asarray(x, dtype=np.float32))
    B = x.shape[0]
    b_core = B // N_CORES
    T = b_core // 128
    gb_generic = not (np.all(np.asarray(gamma) == 1.0)
                      and np.all(np.asarray(beta) == 0.0))
    pm, eb, gb = _host_consts(Wq, bq, Wk, bk, Wv, bv, mask, gamma, beta)
    nc = _get_program(T, gb_generic)

    shards = x.reshape(N_CORES, b_core, D)
    in_maps = []
    for c in range(N_CORES):
        xtil, xr = host_layouts(shards[c], T)
        in_maps.append({
            "xt": xtil, "xr": xr,
            "pm": pm.astype(np.float16), "eb": eb,
            "gb": gb.astype(np.float16),
        })
    res = run_bass_kernel_spmd(nc, in_maps, core_ids=list(range(N_CORES)))
    outs = [unpermute_out(np.asarray(res.results[c]["o"], np.float32), T)
            for c in range(N_CORES)]
    return np.concatenate(outs, axis=0)


# revision 4
# speedup vs baseline: 3.4660x; 1.0818x over previous
"""Trainium2 Bass kernel v5 for nn_AttentionSubModule: PE-centric batched
tiny attention.

Per core: 16384 items = 128 tiles of 128 items; tile = 32 groups x 4 items
(b = 128t + 4g + m). Host pre-permutes x (pure layout staging):
  X~[t, 9m+d', G*j+g...] -> xt[t, 36, (g,j)]: xt[t, 9m+d', 25g+j] = x[b, 9j+d']
  XR[t, 25m+i, 9g+e]   = x[b, 9i+e]
and post-permutes the bf16 output back to [B, 25, 9] f32.

Device pipeline per tile:
 - Projections on PE per token-group h: stationary block-diag-over-m W_h
   [36,36] bf16, moving = xt cols j in h. Evac + per-partition bias ->
   Mq/Mk [36, (j,g)] bf16, MvT [36=(m,e), (j,g)] bf16.
 - v re-layout via DRAM round trip, supertile-batched: MvT -> dmv ->
   Mv [(m,j), (r,e,g)] bf16, ones plane at e=9 (memset) gives Z for free.
 - scores: 128 per-item PE matmuls  lhsT=Mk[9m:9m+9, (j,g=g)] [9,25],
   rhs=Mq[...] [9,25] -> sT psum [(m,j), (g,i)].
 - exp on ACT, bias AP = -8 - 1e9*mask[j] (mask + range shift folded).
 - attn@v: 128 per-item PE matmuls lhsT=ex[(m,j), (g-block i)] [25,25],
   rhs=Mv slice [25,10] -> u psum [(m,i), (g, e-aug)], Z at e=9.
 - residual + LayerNorm in [(m,i), (g,e)] layout on DVE/ACT/Pool
   (LN scale-invariance absorbs the softmax normalizer, as usual).
"""

import numpy as np
from contextlib import ExitStack

import concourse.bass as bass
import concourse.tile as tile
from concourse import mybir
from concourse.bass_utils import run_bass_kernel_spmd

KV = 9
NQ = 25
D = NQ * KV
GROUPS = [(0, 27, 3), (27, 117, 10), (117, 207, 10), (207, 225, 2)]
TOK_H = [(0, 3), (3, 13), (13, 23), (23, 25)]
TOKH_OF = {0: 0, 3: 1, 13: 2, 16: 2, 23: 3}
N_CORES = 8
M4 = 4
G = 32
EPS = 1e-5
F32 = mybir.dt.float32
BF16 = mybir.dt.float16
R_SUP = 8

PM_COLS = 4 * (128 + 128 + 36)
FB_COLS = 12


def build_program_v5(T, gb_generic=False, probe=4):
    assert T % R_SUP == 0
    ST = T // R_SUP
    nc = bass.Bass("TRN2", target_bir_lowering=False)

    xt_d = nc.dram_tensor("xt", [T, 37, G * NQ], BF16, kind="ExternalInput")
    xr_d = nc.dram_tensor("xr", [T, 128, G * KV], BF16, kind="ExternalInput")
    pm_d = nc.dram_tensor("pm", [37, PM_COLS], BF16, kind="ExternalInput")
    eb_d = nc.dram_tensor("eb", [128, 1], F32, kind="ExternalInput")
    gb_d = nc.dram_tensor("gb", [128, 2 * KV], BF16, kind="ExternalInput")
    o_d = nc.dram_tensor("o", [T, 128, G * KV], BF16, kind="ExternalOutput")

    AF = mybir.ActivationFunctionType

    with tile.TileContext(nc) as tc, ExitStack() as ctx:
        consts = ctx.enter_context(tc.tile_pool(name="consts", bufs=1))
        sup = ctx.enter_context(tc.tile_pool(name="sup", bufs=2))
        dram = ctx.enter_context(tc.tile_pool(name="dram", bufs=2, space="DRAM"))
        proj = ctx.enter_context(tc.tile_pool(name="proj", bufs=3))
        expp = ctx.enter_context(tc.tile_pool(name="exsb", bufs=2 * R_SUP + 2))
        lnp = ctx.enter_context(tc.tile_pool(name="lnp", bufs=3))
        pproj = ctx.enter_context(tc.tile_pool(name="pproj", bufs=2, space="PSUM"))
        psc = ctx.enter_context(tc.tile_pool(name="psc", bufs=2, space="PSUM"))
        pu = ctx.enter_context(tc.tile_pool(name="pu", bufs=2, space="PSUM"))

        pm_t = consts.tile([37, PM_COLS], BF16)
        nc.sync.dma_start(out=pm_t, in_=pm_d[:, :])
        eb_t = consts.tile([128, 1], F32)
        nc.sync.dma_start(out=eb_t, in_=eb_d[:, :])
        gb_t = consts.tile([128, 2 * KV], BF16)
        nc.sync.dma_start(out=gb_t, in_=gb_d[:, :])

        Wmat = {}
        for h in range(4):
            c = (128 + 128 + 36) * h
            Wmat["q", h] = pm_t[:, c : c + 128]
            Wmat["k", h] = pm_t[:, c + 128 : c + 256]
            Wmat["v", h] = pm_t[:, c + 256 : c + 292]

        inv3 = float(1.0 / np.sqrt(KV))

        for s in range(ST):
            xts = sup.tile([37, R_SUP * G * NQ], BF16, tag="xts")
            nc.sync.dma_start(
                out=xts[:].rearrange("p (r c) -> p r c", r=R_SUP),
                in_=xt_d[s * R_SUP : (s + 1) * R_SUP, :, :].transpose([1, 0, 2]),
            )
            xrs = sup.tile([128, R_SUP * G * KV], BF16, tag="xrs")
            nc.sync.dma_start(
                out=xrs[:].rearrange("p (r c) -> p r c", r=R_SUP),
                in_=xr_d[s * R_SUP : (s + 1) * R_SUP, :, :].transpose([1, 0, 2]),
            )
            mvts = sup.tile([36, R_SUP * NQ * G], BF16, tag="mvts")
            os_t = sup.tile([128, R_SUP * G * KV], BF16, tag="os")

            exs = []
            # ---------- pass 1: projections, scores, exp ----------
            for r in range(R_SUP):
                xv = xts[:, r * G * NQ : (r + 1) * G * NQ].rearrange(
                    "p (g j) -> p g j", g=G)

                mq_t = proj.tile([128, NQ * G], BF16, tag="mq")
                mk_t = proj.tile([128, NQ * G], BF16, tag="mk")
                mvt = mvts[:, r * NQ * G : (r + 1) * NQ * G]
                # q/k/v sequentially through one 2-bank psum tag.
                # q/k psum layout: col = 25*g + j for g<16, 512 + 25*(g-16) + j
                # v psum layout: col = 32*j + g (contiguous across banks at j=16)
                for nm in ("q", "k", "v"):
                    rows = 36 if nm == "v" else 128
                    pp = pproj.tile([128, 1024], F32, tag="pp")
                    if nm == "v":
                        # split by j at the bank boundary (j=16)
                        for j0, j1 in ((0, 3), (3, 13), (13, 16),
                                       (16, 23), (23, 25)):
                            w = j1 - j0
                            rhs = xts[:, r * G * NQ : (r + 1) * G * NQ].rearrange(
                                "p (g j) -> p g j", g=G)[:, :, j0:j1]
                            dst = bass.AP(
                                tensor=pp[:].tensor,
                                offset=pp[:].offset + 32 * j0,
                                ap=[[pp[:].ap[0][0], rows], [1, G], [G, w]],
                            )
                            nc.tensor.matmul(dst, Wmat[nm, TOKH_OF[j0]], rhs,
                                             start=True, stop=True)
                        nc.scalar.copy(mvt, pp[0:36, 0:800])
                    else:
                        for gh in range(2):
                            for h in range(4):
                                j0, j1 = TOK_H[h]
                                w = j1 - j0
                                rhs = xts[:, r * G * NQ : (r + 1) * G * NQ
                                          ].rearrange("p (g j) -> p g j", g=G)[
                                    :, 16 * gh : 16 * gh + 16, j0:j1]
                                dst = bass.AP(
                                    tensor=pp[:].tensor,
                                    offset=pp[:].offset + 512 * gh + j0,
                                    ap=[[pp[:].ap[0][0], rows], [NQ, 16], [1, w]],
                                )
                                nc.tensor.matmul(dst, Wmat[nm, h], rhs,
                                                 start=True, stop=True)
                        mdst = (mq_t if nm == "q" else mk_t)[:].rearrange(
                            "p (gh c) -> p gh c", gh=2)
                        msrc = bass.AP(
                            tensor=pp[:].tensor, offset=pp[:].offset,
                            ap=[[pp[:].ap[0][0], 128], [512, 2], [1, 400]],
                        )
                        if nm == "q":
                            nc.scalar.copy(mdst, msrc)
                        else:
                            nc.vector.tensor_copy(mdst, msrc)

                if probe <= 1:
                    nc.vector.tensor_copy(
                        os_t[:, r * G * KV : (r + 1) * G * KV],
                        mq_t[:, 0 : G * KV])
                    continue
                # scores: one 2-bank psum, g<16 at cols 25g+j, g>=16 at
                # 512 + 25(g-16) + j
                sc = psc.tile([128, 1024], F32, tag="sc", bufs=1)
                mk3 = mk_t[:].rearrange("p (g j) -> p g j", g=G)
                mq3 = mq_t[:].rearrange("p (g j) -> p g j", g=G)
                for g in range(G):
                    c0 = 25 * g if g < 16 else 512 + 25 * (g - 16)
                    for m in range(M4):
                        nc.tensor.matmul(
                            sc[32 * m : 32 * m + 25, c0 : c0 + 25],
                            mk3[32 * m : 32 * m + 9, g, :],
                            mq3[32 * m : 32 * m + 9, g, :],
                            start=True, stop=True,
                            tile_position=(32 * m, 32 * m))

                ex_t = expp.tile([128, G * NQ], BF16, tag="ex")
                nc.scalar.activation(ex_t[:, 0:400], sc[:, 0:400], AF.Exp,
                                     bias=eb_t[:], scale=inv3)
                nc.scalar.activation(ex_t[:, 400:800], sc[:, 512:912], AF.Exp,
                                     bias=eb_t[:], scale=inv3)
                exs.append(ex_t)
                if probe <= 2:
                    nc.vector.tensor_copy(
                        os_t[:, r * G * KV : (r + 1) * G * KV],
                        ex_t[:, 0 : G * KV])

            if probe <= 2:
                nc.sync.dma_start(
                    out=o_d[s * R_SUP : (s + 1) * R_SUP, :, :].transpose([1, 0, 2]),
                    in_=os_t[:].rearrange("p (r c) -> p r c", r=R_SUP),
                )
                continue
            RH = R_SUP // 4
            for hf in range(4):
                # ---------- v round trip (half supertile) ----------
                # dmv element layout: off = 28800 rh + 1152 j + 288 m + 32 e + g
                dmv = dram.tile([36, RH * NQ * G], BF16, tag=f"dmv{hf}")
                dmv_ap = dmv[:]
                dump_dst = bass.AP(
                    tensor=dmv_ap.tensor, offset=dmv_ap.offset,
                    ap=[[32, 36], [1152, RH * NQ], [1, G]],
                )
                mvh = mvts[:, hf * RH * NQ * G : (hf + 1) * RH * NQ * G]
                nc.sync.dma_start(out=dump_dst, in_=mvh.rearrange(
                    "p (rj g) -> p rj g", g=G))
                mv_s = sup.tile([128, RH * 10 * G], BF16, tag=f"mvs{hf}")
                mv4 = mv_s[:].rearrange("p (r e g) -> p r e g", r=RH, e=10)
                nc.vector.memset(mv4[:, :, 9, :], 1.0)
                for m in range(M4):
                    dstv = mv4[32 * m : 32 * m + 25, :, 0:9, :]
                    srcv = bass.AP(
                        tensor=dmv_ap.tensor,
                        offset=dmv_ap.offset + 288 * m,
                        ap=[[1152, NQ], [28800, RH], [1, KV * G]],
                    )
                    nc.sync.dma_start(out=dstv, in_=srcv)

                # ---------- pass 2 on this half (tile pairs) ----------
                for r2 in range(RH // 2):
                    u_s = lnp.tile([128, 2, 320], F32, tag="us")
                    for half in range(2):
                        rh = 2 * r2 + half
                        r = hf * RH + rh
                        ex_t = exs[r]
                        u_ps = pu.tile([128, 320], F32, tag="u")
                        mv_r = mv4[:, rh, :, :]
                        for g in range(G):
                            e0 = 25 * g if g < 16 else 400 + 25 * (g - 16)
                            for m in range(M4):
                                nc.tensor.matmul(
                                    u_ps[32 * m : 32 * m + 25,
                                         10 * g : 10 * g + 10],
                                    ex_t[32 * m : 32 * m + 25, e0 : e0 + 25],
                                    mv_r[32 * m : 32 * m + 25, :, g],
                                    start=True, stop=True,
                                    tile_position=(32 * m, 32 * m))
                        nc.scalar.copy(u_s[:, half, :], u_ps[:])

                    r = hf * RH + 2 * r2
                    u3 = u_s[:].rearrange("p h (g e) -> p (h g) e", g=G)
                    uvec = u3[:, :, 0:9]
                    z_ap = u3[:, :, 9:10]
                    G2 = 2 * G
                    xr_r = xrs[:, r * G * KV : (r + 2) * G * KV].rearrange(
                        "p (g e) -> p g e", g=G2)

                    zx = lnp.tile([128, G2, KV], F32, tag="zx")
                    nc.gpsimd.tensor_mul(zx[:], xr_r,
                                         z_ap.broadcast_to((128, G2, KV)))
                    u2 = lnp.tile([128, G2, KV], F32, tag="u2")
                    nc.gpsimd.tensor_add(u2[:], uvec, zx[:])
                    s_t = lnp.tile([128, G2], F32, tag="s")
                    nc.vector.tensor_reduce(
                        s_t[:], u2[:], axis=mybir.AxisListType.X,
                        op=mybir.AluOpType.add)
                    mu = lnp.tile([128, G2], F32, tag="mu")
                    nc.gpsimd.tensor_scalar_mul(mu[:], s_t[:], 1.0 / KV)
                    cen = lnp.tile([128, G2, KV], F32, tag="cen")
                    nc.vector.tensor_sub(
                        cen[:], u2[:],
                        mu[:].unsqueeze(2).broadcast_to((128, G2, KV)))
                    sq = lnp.tile([128, G2, KV], F32, tag="sq")
                    nc.gpsimd.tensor_mul(sq[:], cen[:], cen[:])
                    vs = lnp.tile([128, G2], F32, tag="vs")
                    nc.vector.tensor_reduce(
                        vs[:], sq[:], axis=mybir.AxisListType.X,
                        op=mybir.AluOpType.add)
                    zsq = lnp.tile([128, G2], F32, tag="zsq")
                    zf = z_ap.rearrange("p a b -> p (a b)")
                    nc.gpsimd.tensor_mul(zsq[:], zf, zf)
                    vs2 = lnp.tile([128, G2], F32, tag="vs2")
                    nc.vector.scalar_tensor_tensor(
                        vs2[:], zsq[:], float(KV * EPS), vs[:],
                        op0=mybir.AluOpType.mult, op1=mybir.AluOpType.add)
                    sd = lnp.tile([128, G2], F32, tag="sd")
                    nc.scalar.activation(sd[:], vs2[:], AF.Sqrt,
                                         bias=0.0, scale=1.0 / KV)
                    rstd = lnp.tile([128, G2], F32, tag="rstd")
                    nc.vector.reciprocal(rstd[:], sd[:])

                    o_r = os_t[:, r * G * KV : (r + 2) * G * KV].rearrange(
                        "p (g e) -> p g e", g=G2)
                    if gb_generic:
                        o1 = lnp.tile([128, G2, KV], F32, tag="o1")
                        nc.vector.tensor_mul(
                            o1[:], cen[:],
                            rstd[:].unsqueeze(2).broadcast_to((128, G2, KV)))
                        gam = gb_t[:, 0:KV].unsqueeze(1).broadcast_to(
                            (128, G2, KV))
                        bet = gb_t[:, KV : 2 * KV].unsqueeze(1).broadcast_to(
                            (128, G2, KV))
                        o2 = lnp.tile([128, G2, KV], F32, tag="o2")
                        nc.vector.tensor_mul(o2[:], o1[:], gam)
                        nc.vector.tensor_add(o_r, o2[:], bet)
                    else:
                        nc.vector.tensor_mul(
                            o_r, cen[:],
                            rstd[:].unsqueeze(2).broadcast_to((128, G2, KV)))

            nc.sync.dma_start(
                out=o_d[s * R_SUP : (s + 1) * R_SUP, :, :].transpose([1, 0, 2]),
                in_=os_t[:].rearrange("p (r c) -> p r c", r=R_SUP),
            )

    _split_multi_waits(nc)
    return nc


def _split_multi_waits(nc):
    for f in nc.m.functions:
        for b in f.blocks:
            i = 0
            while i < len(b.instructions):
                inst = b.instructions[i]
                si = getattr(inst, "sync_info", None)
                if si is not None and si.on_wait and len(si.on_wait) > 1:
                    extra = si.on_wait[:-1]
                    si.on_wait = si.on_wait[-1:]
                    for w in extra:
                        nop = mybir.InstNoOp(
                            name=nc.get_next_instruction_name(),
                            engine=inst.engine, ins=[], outs=[],
                            sync_info=mybir.SyncInfo(on_wait=[w], on_update=[]),
                        )
                        nc.register_instruction(nop)
                        b.instructions.insert(i, nop)
                        i += 1
                i += 1
    return nc


# ---------------- host side ----------------

def _group_mats(W, b_):
    """Expand per-group (4) mats to per-token-group-h lists."""
    W = np.asarray(W, np.float32)
    b_ = np.asarray(b_, np.float32)
    return [W[h] for h in range(4)], [b_[h] for h in range(4)]


def _host_consts(Wq, bq, Wk, bk, Wv, bv, mask, gamma, beta):
    pm = np.zeros((37, PM_COLS), np.float32)
    for h in range(4):
        c = (128 + 128 + 36) * h
        Wqh = np.asarray(Wq, np.float32)[h]
        Wkh = np.asarray(Wk, np.float32)[h]
        Wvh = np.asarray(Wv, np.float32)[h]
        for m in range(M4):
            pm[9 * m : 9 * m + 9, c + 32 * m : c + 32 * m + 9] = Wqh.T
            pm[9 * m : 9 * m + 9, c + 128 + 32 * m : c + 128 + 32 * m + 9] = Wkh.T
            pm[9 * m : 9 * m + 9, c + 256 + 9 * m : c + 256 + 9 * m + 9] = Wvh.T
            pm[36, c + 32 * m : c + 32 * m + 9] = np.asarray(bq, np.float32)[h]
            pm[36, c + 128 + 32 * m : c + 128 + 32 * m + 9] = np.asarray(
                bk, np.float32)[h]
            pm[36, c + 256 + 9 * m : c + 256 + 9 * m + 9] = np.asarray(
                bv, np.float32)[h]

    eb = np.full((128, 1), -8.0, np.float32)
    mk = np.asarray(mask, np.float32)
    for m in range(M4):
        for j in range(NQ):
            eb[32 * m + j, 0] = -8.0 - 1e9 * float(mk[j])
    gb = np.concatenate([
        np.broadcast_to(np.asarray(gamma, np.float32), (128, KV)),
        np.broadcast_to(np.asarray(beta, np.float32), (128, KV)),
    ], axis=1)
    return pm, eb, np.ascontiguousarray(gb)


def host_layouts(x, T):
    xt4 = x.reshape(T, G, M4, NQ, KV)  # [t, g, m, j, d]
    xtil = np.ones((T, 37, G * NQ), np.float32)
    xtil[:, 0:36] = xt4.transpose(0, 2, 4, 1, 3).reshape(T, 36, G * NQ)
    xr = np.zeros((T, 128, G * KV), np.float16)
    xr4 = xt4.transpose(0, 2, 3, 1, 4).reshape(T, M4, NQ, G * KV)
    for m in range(M4):
        xr[:, 32 * m : 32 * m + 25, :] = xr4[:, m].astype(np.float16)
    return xtil.astype(np.float16), xr


def unpermute_out(o, T):
    o6 = o.reshape(T, 4, 32, G, KV)[:, :, :NQ]  # [t, m, i, g, e]
    o5 = o6.transpose(0, 3, 1, 2, 4)  # [t, g, m, i, e]
    return np.ascontiguousarray(o5).reshape(T * 128, NQ, KV).astype(np.float32)


_NC_CACHE = {}


def _get_program(T, gb_generic):
    key = (T, gb_generic)
    if key not in _NC_CACHE:
        _NC_CACHE[key] = build_program_v5(T, gb_generic)
    return _NC_CACHE[key]


def kernel(x, mask, Wq, bq, Wk, bk, Wv, bv, gamma, beta):
    x = np.ascontiguousarray(np.asarray(x, dtype=np.float32))
    B = x.shape[0]
    b_core = B // N_CORES
    T = b_core // 128
    gb_generic = not (np.all(np.asarray(gamma) == 1.0)
                      and np.all(np.asarray(beta) == 0.0))
    pm, eb, gb = _host_consts(Wq, bq, Wk, bk, Wv, bv, mask, gamma, beta)
    nc = _get_program(T, gb_generic)

    shards = x.reshape(N_CORES, b_core, D)
    in_maps = []
    for c in range(N_CORES):
        xtil, xr = host_layouts(shards[c], T)
        in_maps.append({
            "xt": xtil, "xr": xr,
            "pm": pm.astype(np.float16), "eb": eb,
            "gb": gb.astype(np.float16),
        })
    res = run_bass_kernel_spmd(nc, in_maps, core_ids=list(range(N_CORES)))
    outs = [unpermute_out(np.asarray(res.results[c]["o"], np.float32), T)
            for c in range(N_CORES)]
    return np.concatenate(outs, axis=0)
